# revision 12
# baseline (speedup 1.0000x reference)
"""Euler integrator (low-rank quadratic Christoffel term) on 8 trn2 NeuronCores.

Math: per step   h = v @ U; gamma = (h*h) @ W; v' = v + dt*(force - gamma);
                 x' = wrap(x + dt*v)
Key reduction: the dynamics close in the rank-64 space:
    h_{t+1} = h_t + dt*(force@U) - (h_t^2) @ (dt*W@U)
    v_T = v_0 + T*dt*force - dt * (sum_t h_t^2) @ W
    x_T = wrap(x_0 + T*dt*v_0 + c*dt^2*force - dt^2 * (sum_t (T-1-t) h_t^2) @ W)
with T=8, c = sum_{t<T-1}(T-1-t) = 28.  Only one wrap at the end (mod-2pi
commutes with addition), so HBM traffic is the 5-tensor roofline.
"""

import sys

sys.path.insert(0, "/opt/trn_rl_repo")

import numpy as np
import ml_dtypes

import concourse.bass as bass
import concourse.bacc as bacc
import concourse.mybir as mybir
import concourse.tile as tile
from concourse.tile_rust import add_dep_helper
from concourse.bass_utils import run_bass_kernel_spmd


def _chain(*insts):
    for a, b in zip(insts[1:], insts[:-1]):
        add_dep_helper(a.ins, b.ins, sync=False, reason="psum group order")

F32 = mybir.dt.float32
BF16 = mybir.dt.bfloat16

DT = 0.01
PI = float(np.pi)
TWO_PI = 2.0 * PI
B, D, R = 262144, 256, 64
NCORES = 8
BL = B // NCORES          # rows per core
STEPS = 8
PACK = 1024               # batch rows per pack
NBLK = PACK // 128        # natural 128-row blocks per pack (8)
HN = 512                  # free size of h-space tiles (PACK/2)

# wrap path: "mod" uses DVE mod ALU op; "round" uses magic-number RNE + PE matmul
WRAP_MODE = "round"
MAGIC = 12582912.0        # 1.5 * 2**23


def _build(bl: int):
    npack = bl // PACK
    nc = bacc.Bacc("TRN2", target_bir_lowering=False, debug=False)

    xg = nc.declare_dram_parameter("xg", [bl, D], F32, isOutput=False)
    vg = nc.declare_dram_parameter("vg", [bl, D], F32, isOutput=False)
    fg = nc.declare_dram_parameter("fg", [bl, D], F32, isOutput=False)
    # constants (host-prepared, tiny)
    u0 = nc.declare_dram_parameter("u0", [128, R], F32, isOutput=False)
    u0z = nc.declare_dram_parameter("u0z", [128, 128], F32, isOutput=False)
    u1z = nc.declare_dram_parameter("u1z", [128, 128], F32, isOutput=False)
    u1 = nc.declare_dram_parameter("u1", [128, R], F32, isOutput=False)
    mdn = nc.declare_dram_parameter("mdn", [128, R], BF16, isOutput=False)   # -dt*(W@U), dup'd
    i64 = nc.declare_dram_parameter("i64", [128, R], BF16, isOutput=False)   # I_64, dup'd
    wn = nc.declare_dram_parameter("wn", [128, D], BF16, isOutput=False)     # -dt*W, dup'd
    wnn = nc.declare_dram_parameter("wnn", [128, D], BF16, isOutput=False)   # -dt^2*W, dup'd
    ieye = nc.declare_dram_parameter("ieye", [128, 128], F32, isOutput=False)
    i8 = nc.declare_dram_parameter("i8", [128, 128], F32, isOutput=False)    # 8dt*I
    i28 = nc.declare_dram_parameter("i28", [128, 128], F32, isOutput=False)  # 28dt^2*I
    in2p = nc.declare_dram_parameter("in2p", [128, 128], F32, isOutput=False)  # -2pi*I

    xo = nc.declare_dram_parameter("xo", [bl, D], F32, isOutput=True)
    vo = nc.declare_dram_parameter("vo", [bl, D], F32, isOutput=True)

    A = mybir.AluOpType

    with tile.TileContext(nc) as tc:
        with (
            tc.tile_pool(name="consts", bufs=1) as cpool,
            tc.tile_pool(name="nat", bufs=2) as nat,
            tc.tile_pool(name="natx", bufs=2) as natx,
            tc.tile_pool(name="trans", bufs=2) as trans,
            tc.tile_pool(name="hsp", bufs=3) as hsp,
            tc.tile_pool(name="acc", bufs=2) as accp,
            tc.tile_pool(name="outp", bufs=2) as outp,
            tc.tile_pool(name="wrapp", bufs=2) as wrapp,
            tc.tile_pool(name="ph", bufs=2, space="PSUM") as php,
            tc.tile_pool(name="pt", bufs=2, space="PSUM") as ptp,
            tc.tile_pool(name="pe", bufs=2, space="PSUM") as pep,
        ):
            # ---- constants to SBUF
            u0_s = cpool.tile([128, R], F32, tag="u0")
            u0z_s = cpool.tile([128, 128], F32, tag="u0z")
            u1z_s = cpool.tile([128, 128], F32, tag="u1z")
            u1_s = cpool.tile([128, R], F32, tag="u1")
            mdn_s = cpool.tile([128, R], BF16, tag="mdn")
            i64_s = cpool.tile([128, R], BF16, tag="i64")
            wn_s = cpool.tile([128, D], BF16, tag="wn")
            wnn_s = cpool.tile([128, D], BF16, tag="wnn")
            ieye_s = cpool.tile([128, 128], F32, tag="ieye")
            i8_s = cpool.tile([128, 128], F32, tag="i8")
            i28_s = cpool.tile([128, 128], F32, tag="i28")
            in2p_s = cpool.tile([128, 128], F32, tag="in2p")
            negpi_s = cpool.tile([128, 1], F32, tag="negpi")
            nc.vector.memset(negpi_s[:], -PI)
            for t_, d_ in (
                (u0_s, u0), (u0z_s, u0z), (u1z_s, u1z), (u1_s, u1), (mdn_s, mdn), (i64_s, i64),
                (wn_s, wn), (wnn_s, wnn), (ieye_s, ieye), (i8_s, i8),
                (i28_s, i28), (in2p_s, in2p),
            ):
                nc.sync.dma_start(out=t_[:], in_=d_[:])

            for p in range(npack):
                rows = slice(p * PACK, (p + 1) * PACK)

                # ---- load v, force natural: [128, blk, 256]
                vt = nat.tile([128, NBLK, D], F32, tag="vt")
                ft = nat.tile([128, NBLK, D], F32, tag="ft")
                nc.sync.dma_start(
                    out=vt[:], in_=vg[rows, :].rearrange("(n p) d -> p n d", p=128)
                )
                nc.sync.dma_start(
                    out=ft[:], in_=fg[rows, :].rearrange("(n p) d -> p n d", p=128)
                )

                # ---- transpose v, f -> vT/fT chunks [128(d), 1024(b)]
                vT0 = trans.tile([128, PACK], F32, tag="vT0")
                vT1 = trans.tile([128, PACK], F32, tag="vT1")
                fT0 = trans.tile([128, PACK], F32, tag="fT0")
                fT1 = trans.tile([128, PACK], F32, tag="fT1")
                for src, dsts in ((vt, (vT0, vT1)), (ft, (fT0, fT1))):
                    for dch in range(2):
                        for bg in range(2):
                            ptr = ptp.tile([128, HN], F32, tag="ptr")
                            for j in range(4):
                                blk = bg * 4 + j
                                nc.tensor.transpose(
                                    out=ptr[:, j * 128:(j + 1) * 128],
                                    in_=src[:, blk, dch * 128:(dch + 1) * 128],
                                    identity=ieye_s[:],
                                )
                            nc.scalar.copy(
                                dsts[dch][:, bg * HN:(bg + 1) * HN], ptr[:]
                            )

                # ---- h0 into persistent psum bank; fU -> fUdt (bf16)
                ph = php.tile([128, HN], F32, tag="ph")
                pf = ptp.tile([128, HN], F32, tag="ptr")
                _chain(
                    nc.tensor.matmul(
                        ph[:, :], u0z_s[:, :], vT0[:, 0:HN],
                        start=True, stop=False,
                    ),
                    nc.tensor.matmul(
                        ph[64:128, :], u0_s[:, :], vT0[:, HN:PACK],
                        start=False, stop=False, skip_group_check=True,
                    ),
                    nc.tensor.matmul(
                        ph[64:128, :], u1_s[:, :], vT1[:, HN:PACK],
                        start=False, stop=False, skip_group_check=True,
                    ),
                    nc.tensor.matmul(
                        ph[:, :], u1z_s[:, :], vT1[:, 0:HN],
                        start=False, stop=True,
                    ),
                )
                _chain(
                    nc.tensor.matmul(
                        pf[:, :], u0z_s[:, :], fT0[:, 0:HN],
                        start=True, stop=False,
                    ),
                    nc.tensor.matmul(
                        pf[64:128, :], u0_s[:, :], fT0[:, HN:PACK],
                        start=False, stop=False, skip_group_check=True,
                    ),
                    nc.tensor.matmul(
                        pf[64:128, :], u1_s[:, :], fT1[:, HN:PACK],
                        start=False, stop=False, skip_group_check=True,
                    ),
                    nc.tensor.matmul(
                        pf[:, :], u1z_s[:, :], fT1[:, 0:HN],
                        start=False, stop=True,
                    ),
                )
                fUdt = hsp.tile([128, HN], BF16, tag="fUdt")
                nc.scalar.mul(fUdt[:], pf[:], DT)

                # ---- step loop: square in ACT, A/G accumulate on DVE,
                #      h-update via accumulating matmuls into ph
                Aacc = accp.tile([128, HN], BF16, tag="Aacc")
                Gacc = accp.tile([128, HN], BF16, tag="Gacc")
                for t in range(STEPS):
                    hsq = hsp.tile([128, HN], BF16, tag="hsq")
                    nc.scalar.square(hsq[:], ph[:])
                    if t == 0:
                        nc.vector.tensor_copy(Aacc[:], hsq[:])
                        nc.vector.tensor_copy(Gacc[:], hsq[:])
                    else:
                        nc.vector.tensor_tensor(Aacc[:], Aacc[:], hsq[:], A.add)
                        if t <= STEPS - 2:
                            nc.vector.tensor_tensor(Gacc[:], Gacc[:], Aacc[:], A.add)
                    if t < STEPS - 1:
                        last = t == STEPS - 2
                        for half in range(2):
                            osl = slice(half * 64, (half + 1) * 64)
                            nc.tensor.matmul(
                                ph[osl, :], mdn_s[osl, :], hsq[osl, :],
                                start=False, stop=False,
                                skip_group_check=True,
                            )
                            nc.tensor.matmul(
                                ph[osl, :], i64_s[osl, :], fUdt[osl, :],
                                start=False, stop=False,
                                skip_group_check=True,
                            )

                # ---- epilogue
                xt = natx.tile([128, NBLK, D], F32, tag="xt")
                nc.sync.dma_start(
                    out=xt[:], in_=xg[rows, :].rearrange("(n p) d -> p n d", p=128)
                )
                vf_sb = outp.tile([128, NBLK, D], F32, tag="vf_sb")
                xf_sb = outp.tile([128, NBLK, D], F32, tag="xf_sb")

                for bg in range(4):           # bank groups: 2 natural blocks each
                    pvf = pep.tile([128, 2, D], F32, tag="pvf")
                    pxf = pep.tile([128, 2, D], F32, tag="pxf")
                    vf_mms = []
                    xf_mms = []
                    for j in range(2):
                        blk = bg * 2 + j
                        half = blk // 4
                        hsl = slice(half * 64, (half + 1) * 64)
                        lsl = slice((blk % 4) * 128, (blk % 4) * 128 + 128)
                        # vf: A@(-dt W) + v0   (+ 8dt*force via DVE stt below)
                        vf_mms.append(nc.tensor.matmul(
                            pvf[:, j, :], Aacc[hsl, lsl], wn_s[hsl, :],
                            start=(j == 0), stop=False,
                        ))
                        vf_mms.append(nc.tensor.matmul(
                            pvf[:, j, :], ieye_s[:], vt[:, blk, :],
                            start=False, stop=(j == 1),
                        ))
                        # xf: G@(-dt^2 W) + x0 + 8dt*v0 + 28dt^2*force
                        xf_mms.append(nc.tensor.matmul(
                            pxf[:, j, :], Gacc[hsl, lsl], wnn_s[hsl, :],
                            start=(j == 0), stop=False,
                        ))
                        xf_mms.append(nc.tensor.matmul(
                            pxf[:, j, :], ieye_s[:], xt[:, blk, :],
                            start=False, stop=False,
                        ))
                        xf_mms.append(nc.tensor.matmul(
                            pxf[:, j, :], i8_s[:], vt[:, blk, :],
                            start=False, stop=False,
                        ))
                        xf_mms.append(nc.tensor.matmul(
                            pxf[:, j, :], i28_s[:], ft[:, blk, :],
                            start=False, stop=(j == 1),
                        ))
                    _chain(*vf_mms)
                    _chain(*xf_mms)
                    b0, b1 = bg * 2, bg * 2 + 2
                    # vf = 8dt*force + pvf
                    nc.vector.scalar_tensor_tensor(
                        out=vf_sb[:, b0:b1, :],
                        in0=ft[:, b0:b1, :],
                        scalar=8.0 * DT,
                        in1=pvf[:],
                        op0=A.mult,
                        op1=A.add,
                    )
                    if WRAP_MODE == "mod":
                        # wrap: ((y + 5pi) mod 2pi) - pi
                        wtmp = wrapp.tile([128, 2, D], F32, tag="wtmp")
                        nc.vector.tensor_scalar(
                            wtmp[:], pxf[:], 5.0 * PI, TWO_PI, A.add, A.mod,
                        )
                        nc.scalar.activation(
                            out=xf_sb[:, b0:b1, :],
                            in_=wtmp[:],
                            func=mybir.ActivationFunctionType.Identity,
                            bias=negpi_s[:], scale=1.0,
                        )
                    else:
                        # wrap via RNE magic number: r = RNE(y/2pi); y -= 2pi*r
                        a1 = wrapp.tile([128, 2, D], F32, tag="a1")
                        nc.vector.tensor_scalar(
                            a1[:], pxf[:], 1.0 / TWO_PI, MAGIC, A.mult, A.add,
                        )
                        rr = wrapp.tile([128, 2, D], F32, tag="rr")
                        nc.vector.tensor_scalar(
                            rr[:], a1[:], MAGIC, None, A.subtract,
                        )
                        for j in range(2):
                            nc.tensor.matmul(
                                pxf[:, j, :], in2p_s[:], rr[:, j, :],
                                start=False, stop=False,
                                skip_group_check=True,
                            )
                        nc.scalar.copy(xf_sb[:, b0:b1, :], pxf[:])

                nc.sync.dma_start(
                    out=vo[rows, :].rearrange("(n p) d -> p n d", p=128),
                    in_=vf_sb[:],
                )
                nc.sync.dma_start(
                    out=xo[rows, :].rearrange("(n p) d -> p n d", p=128),
                    in_=xf_sb[:],
                )

    nc.compile()
    return nc


_NC_CACHE = {}


def _get_nc(bl: int):
    if bl not in _NC_CACHE:
        _NC_CACHE[bl] = _build(bl)
    return _NC_CACHE[bl]


def _consts(U, W):
    U32 = np.ascontiguousarray(U, dtype=np.float32)
    W32 = np.ascontiguousarray(W, dtype=np.float32)
    bf = ml_dtypes.bfloat16
    dup = lambda a: np.concatenate([a, a], axis=0)
    md = -(DT * (W32 @ U32))                         # [64, 64]
    eye = np.eye(128, dtype=np.float32)
    return {
        "u0": U32[:128, :],
        "u0z": np.concatenate(
            [U32[:128, :], np.zeros((128, 64), np.float32)], axis=1
        ),
        "u1z": np.concatenate(
            [U32[128:, :], np.zeros((128, 64), np.float32)], axis=1
        ),
        "u1": U32[128:, :],
        "mdn": dup(md).astype(bf),
        "i64": dup(np.eye(R, dtype=np.float32)).astype(bf),
        "wn": dup(-DT * W32).astype(bf),
        "wnn": dup(-DT * DT * W32).astype(bf),
        "ieye": eye,
        "i8": (8.0 * DT) * eye,
        "i28": (28.0 * DT * DT) * eye,
        "in2p": (-TWO_PI) * eye,
    }


def kernel(x, v, force, U, W, steps=STEPS, **_ignored):
    assert int(steps) == STEPS, f"kernel hardcodes steps={STEPS}, got {steps}"
    x = np.ascontiguousarray(x, dtype=np.float32)
    v = np.ascontiguousarray(v, dtype=np.float32)
    force = np.ascontiguousarray(force, dtype=np.float32)
    consts = _consts(U, W)

    nc = _get_nc(BL)
    in_maps = []
    for i in range(NCORES):
        sl = slice(i * BL, (i + 1) * BL)
        m = {"xg": x[sl], "vg": v[sl], "fg": force[sl]}
        m.update(consts)
        in_maps.append(m)

    res = run_bass_kernel_spmd(nc, in_maps, core_ids=list(range(NCORES)))
    xf = np.concatenate([res.results[i]["xo"] for i in range(NCORES)], axis=0)
    vf = np.concatenate([res.results[i]["vo"] for i in range(NCORES)], axis=0)
    return (xf, vf)
